# revision 1
# baseline (speedup 1.0000x reference)
"""Multi-head attention forward (B=2, N=2048, C=768, H=12) on 8 TRN2 cores.

Sharding: core = b*4 + g handles batch b, heads 3g..3g+2 (tensor parallel on
heads). Each core computes qkv for its heads, flash-style attention with the
full N x N logits kept on-chip (transposed [m, nq] layout so the key mask
folds into the exp bias and the softmax denominator comes from ones-columns
in V riding the PV matmul), and a partial output projection over its 192
channels. Host sums the 4 partials per batch and adds the bias.

All matmuls run in float32r (TF32-like, 1 cycle/row at free-dim >= 256).
K=64 logits matmuls for head pairs are row-packed onto disjoint PE row
groups (partitions 0:64 vs 64:128) so they run concurrently; head 2 gets
q/k duplicated into both partition halves so its two nq-halves pack the
same way. Head 1's V block is [ones | v] so its attention rows land on
psum partitions 64:128, making the whole normalize chain lane-local and
letting heads 0+1 share one K=128 projection matmul.
"""

import numpy as np

from concourse import bacc
import concourse.mybir as mybir
import concourse.tile as tile
from concourse.bass_utils import run_bass_kernel_spmd

B, N, C = 2, 2048, 768
H, DH = 12, 64
G = 4          # head groups (cores per batch)
HPC = 3        # heads per core
P = 128
KT = C // P    # 6 contraction tiles over channels
NMT = N // P   # 16 key (m) tiles
NQT = N // P   # 16 query tiles
W = 1024       # nq chunk width for logits/exp
NH = N // W    # 2 nq chunks
VBLK = 2 * DH  # 128: per-head lhsT width ([v|1] or [1|v])
VB = 5 * DH    # 320 per m-tile: [v0 | ones | v1 | ones | v2]

TRACE = False
LAST_EXEC_NS = None
LAST_RESULTS = None

_nc_cache = {}

f32 = mybir.dt.float32
f32r = mybir.dt.float32r

_VONES = np.ones((P, NMT * 2 * DH), np.float32)


def _build(reps=1):
    nc = bacc.Bacc("TRN2", debug=False)

    xT = nc.dram_tensor("xT", [C, N], f32r, kind="ExternalInput")
    wqkT = nc.dram_tensor("wqkT", [C, 6 * DH], f32r, kind="ExternalInput")
    wvT = nc.dram_tensor("wvT", [C, VB], f32r, kind="ExternalInput")
    wpT = nc.dram_tensor("wpT", [HPC * DH, C], f32r, kind="ExternalInput")
    mbias = nc.dram_tensor("mbias", [P, NMT], f32, kind="ExternalInput")
    vones = nc.dram_tensor("vones", [P, NMT * 2 * DH], f32r, kind="ExternalInput")
    y = nc.dram_tensor("y", [N, C], f32, kind="ExternalOutput")

    with tile.TileContext(nc) as tc:
        with (
            tc.tile_pool(name="big", bufs=1) as big,
            tc.tile_pool(name="exps", bufs=8) as exps,
            tc.tile_pool(name="recips", bufs=2) as recips,
            tc.tile_pool(name="ys", bufs=6) as ys,
            tc.tile_pool(name="pa", bufs=2, space="PSUM") as pa,
            tc.tile_pool(name="pb", bufs=2, space="PSUM") as pb,
        ):
            body(nc, tc, big, exps, recips, ys, pa, pb,
                 xT, wqkT, wvT, wpT, mbias, vones, y, reps)

    nc.compile()
    return nc


def body(nc, tc, big, exps, recips, ys, pa, pb,
         xT, wqkT, wvT, wpT, mbias, vones, y, reps):
    QC = HPC * DH  # 192: q block width in wqkT
    for _rep in range(reps):
        xT_sb = big.tile([P, KT * N], f32r, tag="xT", name="xT_sb")
        wqk_sb = big.tile([P, KT * 6 * DH], f32r, tag="wqk", name="wqk_sb")
        wv_sb = big.tile([P, KT * VB], f32r, tag="wv", name="wv_sb")
        wpA = big.tile([P, C], f32r, tag="wpA", name="wpA")   # heads 0+1
        wpB = big.tile([P, C], f32r, tag="wpB", name="wpB")   # head 2 rows 64:128
        mb_sb = big.tile([P, NMT], f32, tag="mb", name="mb_sb")
        ones_sb = big.tile([P, P], f32r, tag="ones", name="ones_sb")
        # t1: q/k for heads 0 (parts 0:64) and 1 (parts 64:128)
        # t2: q/k for head 2, duplicated into both partition halves
        t1 = big.tile([P, 2 * N], f32r, tag="t1", name="t1")
        t2 = big.tile([P, 2 * N], f32r, tag="t2", name="t2")
        v_sb = big.tile([P, NMT * VB], f32r, tag="v", name="v_sb")
        atA = big.tile([P, N], f32r, tag="atA", name="atA")   # h0 rows 0:64, h1 rows 64:128
        atB = big.tile([P, N], f32r, tag="atB", name="atB")   # h2 rows 64:128

        # --- input DMAs (coalesced; ordered by first use, xT first) ---
        nc.sync.dma_start(
            wqk_sb[:].rearrange("p (k c) -> p k c", c=6 * DH),
            wqkT[:, :].rearrange("(k p) c -> p k c", p=P),
        )
        xTv = xT[:, :].rearrange("(k p) n -> p k n", p=P)
        xsv = xT_sb[:].rearrange("p (k n) -> p k n", n=N)
        nc.sync.dma_start(xsv[:, 0:2, :], xTv[:, 0:2, :])
        nc.sync.dma_start(xsv[:, 2:4, :], xTv[:, 2:4, :])
        nc.sync.dma_start(xsv[:, 4:6, :], xTv[:, 4:6, :])
        nc.sync.dma_start(
            wv_sb[:].rearrange("p (k c) -> p k c", c=VB),
            wvT[:, :].rearrange("(k p) c -> p k c", p=P),
        )
        nc.sync.dma_start(mb_sb[:], mbias[:, :])
        nc.sync.dma_start(ones_sb[:], vones[:, 0:P])
        # ones regions of v_sb: cols 64:128 and 192:256 of each 320 block
        vview = v_sb[:].rearrange("p (m q) -> p m q", q=VB)
        nc.sync.dma_start(vview[:, :, DH : 2 * DH], vones[:, 0 : NMT * DH])
        nc.sync.dma_start(
            vview[:, :, 3 * DH : 4 * DH], vones[:, NMT * DH : 2 * NMT * DH]
        )
        nc.sync.dma_start(wpA[:], wpT[0 : 2 * DH, :])
        nc.sync.dma_start(wpB[DH:P, :], wpT[2 * DH : 3 * DH, :])

        # --- qT/kT in d-major layout ---
        def qk_pass(c0, copies, chunks=None, pool=None):
            # copies: list of (psum row range, dest tile, dest col offset)
            for ch in chunks if chunks is not None else range(N // W):
                ps = (pool or pa).tile(
                    [P, W], f32, tag="pb" if pool else "pa", name="ps_qk"
                )
                for s in range(W // 512):
                    for k in range(KT):
                        nc.tensor.matmul(
                            ps[:, s * 512 : (s + 1) * 512],
                            wqk_sb[:, k * 6 * DH + c0 : k * 6 * DH + c0 + P],
                            xT_sb[
                                :,
                                k * N + ch * W + s * 512 : k * N
                                + ch * W
                                + (s + 1) * 512,
                            ],
                            start=(k == 0),
                            stop=(k == KT - 1),
                        )
                for r0, r1, dest, dcol in copies:
                    nc.vector.tensor_copy(
                        dest[r0:r1, dcol + ch * W : dcol + (ch + 1) * W],
                        ps[r0:r1, :],
                    )


        # --- v in natural [m, d] layout ---
        def v_tile(mt, pool=None):
            ps = (pool or pb).tile([P, W], f32, tag="pa" if pool else "pb", name="ps_v")
            for k in range(KT):
                nc.tensor.matmul(
                    ps[:, :VB],
                    xT_sb[:, k * N + mt * P : k * N + (mt + 1) * P],
                    wv_sb[:, k * VB : (k + 1) * VB],
                    start=(k == 0),
                    stop=(k == KT - 1),
                )
            for c0 in (0, 2 * DH, 4 * DH):
                nc.vector.tensor_copy(
                    v_sb[:, mt * VB + c0 : mt * VB + c0 + DH],
                    ps[:, c0 : c0 + DH],
                )

        # per-head PV lhsT within m-tile block [v0 |1| v1 |1| v2]:
        # h0 = [v0|ones] (lo), h1 = [ones|v1] (hi), h2 = [ones|v2] (hi)
        def vap(h, mt):
            off = mt * VB + (0 if h == 0 else DH if h == 1 else 3 * DH)
            return v_sb[:, off : off + VBLK]

        # one softmax stream: two row-packed (lo/hi partition) attention
        # pipelines over the 16 key tiles, then lane-local normalize.
        # units: (qk tile, q col, v block col, out AP, flavor)
        # flavor 'lo': v block [v|1] -> attn rows 0:64, s row 64
        # flavor 'hi': v block [1|v] -> s row 0, attn rows 64:128
        def attn_stream(u_lo, u_hi, per_mt=None, tail=False, defer_norm=False):
            ps_lo = pb.tile([P, W], f32, tag="pb", name="ps_pv0")
            ps_hi = pb.tile([P, W], f32, tag="pb", name="ps_pv1")
            for mt in range(NMT):
                if per_mt is not None:
                    per_mt(mt)
                ets = []
                for (qk, qcol, vh, out_ap, flavor), prow in ((u_lo, 0), (u_hi, DH)):
                    ps_l = pa.tile([P, W], f32, tag="pa", name="ps_l")
                    for s in range(W // 512):
                        nc.tensor.matmul(
                            ps_l[:, s * 512 : (s + 1) * 512],
                            qk[prow : prow + DH, N + mt * P : N + (mt + 1) * P],
                            qk[prow : prow + DH, qcol + s * 512 : qcol + (s + 1) * 512],
                            start=True,
                            stop=True,
                        )
                    et = exps.tile([P, W], f32r, tag="exp", name="et")
                    nc.scalar.activation(
                        et[:],
                        ps_l[:],
                        mybir.ActivationFunctionType.Exp,
                        bias=mb_sb[:, mt : mt + 1],
                        scale=float(DH) ** -0.5,
                    )
                    ets.append(et)
                for et, ps_pv, (qk, qcol, vh, out_ap, flavor) in (
                    (ets[0], ps_lo, u_lo),
                    (ets[1], ps_hi, u_hi),
                ):
                    for s in range(W // 512):
                        nc.tensor.matmul(
                            ps_pv[:, s * 512 : (s + 1) * 512],
                            vap(vh, mt),
                            et[:, s * 512 : (s + 1) * 512],
                            start=(mt == 0),
                            stop=(mt == NMT - 1),
                        )
            # normalize (optionally deferred so the next stream's logits
            # claim psum slots before the norm's broadcast tile)
            def normalize():
              for ps_pv, (qk, qcol, vh, out_ap, flavor) in (
                (ps_lo, u_lo),
                (ps_hi, u_hi),
              ):
                rc = recips.tile([P, W], f32r, tag="rc", name="rc")
                ps_rb = pa.tile([P, W], f32, tag="pa", name="ps_rb")
                rb = recips.tile([P, W], f32, tag="rb", name="rb")
                if flavor == "lo":
                    srow, arow = DH, 0
                else:
                    srow, arow = 0, DH
                with nc.allow_low_precision(reason="f32r softmax denom"):
                    nc.vector.reciprocal(
                        rc[srow : srow + 1, :], ps_pv[srow : srow + 1, :]
                    )
                for s in range(W // 512):
                    nc.tensor.matmul(
                        ps_rb[:, s * 512 : (s + 1) * 512],
                        ones_sb[srow : srow + 1, :],
                        rc[srow : srow + 1, s * 512 : (s + 1) * 512],
                        start=True,
                        stop=True,
                    )
                if tail:
                    nc.scalar.copy(
                        rb[arow : arow + DH, :], ps_rb[arow : arow + DH, :]
                    )
                else:
                    nc.vector.tensor_copy(
                        rb[arow : arow + DH, :], ps_rb[arow : arow + DH, :]
                    )
                nc.vector.tensor_mul(
                    out_ap,
                    ps_pv[arow : arow + DH, :],
                    rb[arow : arow + DH, :],
                )

            if defer_norm:
                return normalize
            normalize()
            return None

        def proj(nt):
            ps_y = pa.tile([P, W], f32, tag="pa", name="ps_y")
            for o0, ow in ((0, 512), (512, 256)):
                nc.tensor.matmul(
                    ps_y[:, o0 : o0 + ow],
                    atA[:, nt * P : (nt + 1) * P],
                    wpA[:, o0 : o0 + ow],
                    start=True,
                    stop=False,
                )
                nc.tensor.matmul(
                    ps_y[:, o0 : o0 + ow],
                    atB[DH:P, nt * P : (nt + 1) * P],
                    wpB[DH:P, o0 : o0 + ow],
                    start=False,
                    stop=True,
                )
            yt = ys.tile([P, C], f32, tag="y", name="yt")
            if nt % 2 == 0:
                nc.vector.tensor_copy(yt[:], ps_y[:, :C])
            else:
                nc.scalar.copy(yt[:], ps_y[:, :C])
            nc.sync.dma_start(y[nt * P : (nt + 1) * P, :], yt[:])

        qk_pass(0, [(0, P, t1, 0)], chunks=[0])          # q heads 0,1
        qk_pass(0, [(0, P, t1, 0)], chunks=[1], pool=pb)
        qk_pass(P, [(0, P, t1, N)], chunks=[0])          # k heads 0,1
        qk_pass(P, [(0, P, t1, N)], chunks=[1], pool=pb)
        for mt in range(NMT):
            v_tile(mt)
        # q2/k2: rows 0:64 = q2 -> t2 lo/q, rows 64:128 = k2 -> t2 hi/k;
        # the two SBUF->SBUF DMAs fill the other partition halves
        qk_pass(2 * P, [(0, DH, t2, 0), (DH, P, t2, N)], pool=pb)
        nc.sync.dma_start(t2[DH:P, 0:N], t2[0:DH, 0:N])      # q2 -> hi
        nc.sync.dma_start(t2[0:DH, N : 2 * N], t2[DH:P, N : 2 * N])  # k2 -> lo

        # heads 0+1, first nq half (normalize deferred into h2's stream)
        norm0 = attn_stream(
            (t1, 0, 0, atA[0:DH, 0:W], "lo"),
            (t1, 0, 1, atA[DH:P, 0:W], "hi"),
            defer_norm=True,
        )

        # head 2: both nq halves at once via the duplicated q2/k2 rows
        norm2 = attn_stream(
            (t2, 0, 2, atB[DH:P, 0:W], "hi"),
            (t2, W, 2, atB[DH:P, W : 2 * W], "hi"),
            per_mt=lambda mt: norm0() if mt == 1 else None,
            defer_norm=True,
        )

        # heads 0+1 second half: h2's deferred normalize fires early, then
        # half-0's projection tiles interleave so they overlap this stream
        def half1_hook(mt):
            if mt == 1:
                norm2()
            elif 3 <= mt < 3 + NQT // NH:
                proj(mt - 3)

        attn_stream(
            (t1, W, 0, atA[0:DH, W : 2 * W], "lo"),
            (t1, W, 1, atA[DH:P, W : 2 * W], "hi"),
            per_mt=half1_hook,
            tail=True,
        )
        for nt in range(NQT // NH, NQT):
            proj(nt)


def _get_nc(reps=1):
    if reps not in _nc_cache:
        _nc_cache[reps] = _build(reps)
    return _nc_cache[reps]


def kernel(x, att_mask, qkv_w, proj_w, proj_b):
    global LAST_EXEC_NS, LAST_RESULTS
    x = np.asarray(x, dtype=np.float32)
    att_mask = np.asarray(att_mask)
    qkv_w = np.asarray(qkv_w, dtype=np.float32)
    proj_w = np.asarray(proj_w, dtype=np.float32)
    proj_b = np.asarray(proj_b, dtype=np.float32)

    nc = _get_nc()

    in_maps = []
    for b in range(B):
        xT = np.ascontiguousarray(x[b].T)
        mb = np.where(att_mask[b] == 0, -1e30, 0.0).astype(np.float32)
        mbias = np.ascontiguousarray(mb.reshape(NMT, P).T)
        for g in range(G):
            r0 = g * HPC * DH
            r1 = (g + 1) * HPC * DH
            wq = qkv_w[r0:r1]                # [192, 768]
            wk = qkv_w[C + r0 : C + r1]
            wv = qkv_w[2 * C + r0 : 2 * C + r1]
            wqkT = np.ascontiguousarray(
                np.concatenate(
                    [wq[0 : 2 * DH], wk[0 : 2 * DH], wq[2 * DH :], wk[2 * DH :]], 0
                ).T
            )
            wvT = np.zeros((C, VB), np.float32)
            wvT[:, 0:DH] = wv[0:DH].T              # v h0 at block col 0
            wvT[:, 2 * DH : 3 * DH] = wv[DH : 2 * DH].T   # v h1 at 128:192
            wvT[:, 4 * DH : 5 * DH] = wv[2 * DH :].T      # v h2 at 256:320
            wpT = np.ascontiguousarray(proj_w[:, r0:r1].T)
            in_maps.append(
                {
                    "xT": xT,
                    "wqkT": wqkT,
                    "wvT": wvT,
                    "wpT": wpT,
                    "mbias": mbias,
                    "vones": _VONES,
                }
            )

    res = run_bass_kernel_spmd(
        nc, in_maps, core_ids=list(range(B * G)), trace=TRACE
    )
    LAST_EXEC_NS = res.exec_time_ns
    LAST_RESULTS = res

    out = np.zeros((B, N, C), np.float32)
    for b in range(B):
        acc = res.results[b * G]["y"].copy()
        for g in range(1, G):
            acc += res.results[b * G + g]["y"]
        out[b] = acc + proj_b[None, :]
    return out



# revision 7
# speedup vs baseline: 1.0642x; 1.0642x over previous
"""Multi-head attention forward (B=2, N=2048, C=768, H=12) on 8 TRN2 cores.

Sharding: core = b*4 + g handles batch b, heads 3g..3g+2. Each core computes
qkv for its heads (all matmul operands bf16), full N x N logits per head in
[key, query] orientation (key mask folds into the exp bias), exp on the
Activation engine into bf16 SBUF tiles, then a "flipped" PV: the exp tile is
the stationary operand and the moving operand is the 65-wide [v | ones]
block, so each 128-query chain costs 65 cycles/k-tile and the softmax
denominator rides in column 64. Normalization is a per-partition
reciprocal+scale on DVE; a single xbar DMA transpose per (head-pair,
query-half) flips the [query, dim] chains into the d-major layout the output
projection needs. Host sums the 4 per-group partial projections per batch
and adds the bias.

Work is laid out as 6 sequential units (3 heads x 2 query halves). Unit U's
logits/exp loop is interleaved (in PE program order) with unit U-1's PV
chains plus qkv/v/proj filler passes so the PE never waits on the
Activation engine, which is the per-unit long pole.
"""

import numpy as np
import ml_dtypes

from concourse import bacc
import concourse.mybir as mybir
import concourse.tile as tile
from concourse.bass_utils import run_bass_kernel_spmd

B, N, C = 2, 2048, 768
H, DH = 12, 64
G = 4           # head groups (cores per batch)
HPC = 3         # heads per core
P = 128
KT = C // P     # 6 contraction tiles over channels
NMT = N // P    # 16 key tiles
W = 1024        # query-half width
NQT = N // P    # 16 query tiles (128 each)
JT = W // P     # 8 query tiles per half
VB = HPC * (DH + 1)   # 195: per-mt v block [v0|1|v1|1|v2|1]
SCALE = float(DH) ** -0.5

TRACE = False
LAST_EXEC_NS = None
LAST_RESULTS = None

_nc_cache = {}

f32 = mybir.dt.float32
bf16 = mybir.dt.bfloat16


def _build(reps=1):
    nc = bacc.Bacc("TRN2", debug=False)

    xT = nc.dram_tensor("xT", [C, N], bf16, kind="ExternalInput")
    wqkT = nc.dram_tensor("wqkT", [C, 3 * P], bf16, kind="ExternalInput")
    wvT = nc.dram_tensor("wvT", [C, HPC * DH], bf16, kind="ExternalInput")
    wpTA = nc.dram_tensor("wpTA", [P, C], bf16, kind="ExternalInput")
    wpTB = nc.dram_tensor("wpTB", [DH, C], bf16, kind="ExternalInput")
    mbias = nc.dram_tensor("mbias", [P, NMT], f32, kind="ExternalInput")
    y = nc.dram_tensor("y", [N, C], bf16, kind="ExternalOutput")

    with tile.TileContext(nc) as tc:
        with (
            tc.tile_pool(name="big", bufs=1) as big,
            tc.tile_pool(name="exps", bufs=34) as exps,
            tc.tile_pool(name="norms", bufs=2) as norms,
            tc.tile_pool(name="rcs", bufs=6) as rcs,
            tc.tile_pool(name="ys", bufs=4) as ys,
            tc.tile_pool(name="pa", bufs=2, space="PSUM") as pa,
            tc.tile_pool(name="ch", bufs=4, space="PSUM") as ch,
        ):
            for _ in range(reps):
                body(nc, tc, big, exps, norms, rcs, ys, pa, ch,
                     xT, wqkT, wvT, wpTA, wpTB, mbias, y)

    nc.compile()
    return nc


def body(nc, tc, big, exps, norms, rcs, ys, pa, ch,
         xT, wqkT, wvT, wpTA, wpTB, mbias, y):
    xT_sb = big.tile([P, KT * N], bf16, tag="xT", name="xT_sb")
    wqk_sb = big.tile([P, KT * 3 * P], bf16, tag="wqk", name="wqk_sb")
    wv_sb = big.tile([P, KT * HPC * DH], bf16, tag="wv", name="wv_sb")
    wpA = big.tile([P, C], bf16, tag="wpA", name="wpA")
    wpB = big.tile([DH, C], bf16, tag="wpB", name="wpB")
    mb_sb = big.tile([P, NMT], f32, tag="mb", name="mb_sb")
    qA = big.tile([P, N], bf16, tag="qA", name="qA")    # q d-major h0|h1
    kA = big.tile([P, N], bf16, tag="kA", name="kA")    # k d-major h0|h1
    qB = big.tile([DH, N], bf16, tag="qB", name="qB")   # q d-major h2
    kB = big.tile([DH, N], bf16, tag="kB", name="kB")   # k d-major h2
    v_sb = big.tile([P, NMT * VB], bf16, tag="v", name="v_sb")
    atA = big.tile([P, N], bf16, tag="atA", name="atA")  # d-major attn h0|h1
    atB = big.tile([DH, N], bf16, tag="atB", name="atB")  # d-major attn h2

    # --- input DMAs (x in 4 query chunks so compute starts early) ---
    xTv = xT[:, :].rearrange("(k p) n -> p k n", p=P)
    xsv = xT_sb[:].rearrange("p (k n) -> p k n", n=N)
    XC = 4
    for c in range(XC):
        nc.sync.dma_start(
            xsv[:, :, c * (N // XC) : (c + 1) * (N // XC)],
            xTv[:, :, c * (N // XC) : (c + 1) * (N // XC)],
        )
    nc.sync.dma_start(
        wqk_sb[:].rearrange("p (k c) -> p k c", c=3 * P),
        wqkT[:, :].rearrange("(k p) c -> p k c", p=P),
    )
    nc.sync.dma_start(
        wv_sb[:].rearrange("p (k c) -> p k c", c=HPC * DH),
        wvT[:, :].rearrange("(k p) c -> p k c", p=P),
    )
    nc.sync.dma_start(mb_sb[:], mbias[:, :])
    nc.sync.dma_start(wpA[:], wpTA[:, :])
    nc.sync.dma_start(wpB[:], wpTB[:, :])

    # ones columns of the v blocks (static)
    nc.gpsimd.memset(
        v_sb[:].rearrange("p (a c) -> p a c", c=DH + 1)[:, :, DH : DH + 1], 1.0
    )

    # --- qk pass: d-major q/k for one 512-query chunk ---
    # wqk col blocks: [wq_h0|wq_h1][wk_h0|wk_h1][wq_h2|wk_h2]
    def qk_pass(col0, rows, c, dest):
        ps = pa.tile([P, 512], f32, tag="pa", name="ps_qk")
        for kt in range(KT):
            nc.tensor.matmul(
                ps[0:rows, :],
                wqk_sb[:, kt * 3 * P + col0 : kt * 3 * P + col0 + rows],
                xT_sb[:, kt * N + c * 512 : kt * N + (c + 1) * 512],
                start=(kt == 0),
                stop=(kt == KT - 1),
            )
        nc.vector.tensor_copy(dest[:, c * 512 : (c + 1) * 512], ps[0:rows, :])

    # --- v pass: keys-major v for one 128-key tile ---
    def v_tile(mt):
        ps = pa.tile([P, HPC * DH], f32, tag="pa", name="ps_v")
        for kt in range(KT):
            nc.tensor.matmul(
                ps[:, :],
                xT_sb[:, kt * N + mt * P : kt * N + (mt + 1) * P],
                wv_sb[:, kt * HPC * DH : (kt + 1) * HPC * DH],
                start=(kt == 0),
                stop=(kt == KT - 1),
            )
        nc.vector.tensor_copy(
            v_sb[:].rearrange("p (m a c) -> p (m a) c", c=DH + 1, a=HPC)[
                :, mt * HPC : (mt + 1) * HPC, 0:DH
            ],
            ps[:].rearrange("p (a c) -> p a c", c=DH),
        )

    # q/k APs per head: (tile, row0)
    QAP = {0: (qA, 0), 1: (qA, DH), 2: (qB, 0)}
    KAP = {0: (kA, 0), 1: (kA, DH), 2: (kB, 0)}

    def logits(h, w, mt):
        qt, qr = QAP[h]
        kt_, kr = KAP[h]
        ps = pa.tile([P, W], f32, tag="pa", name="ps_l")
        for s in range(W // 512):
            nc.tensor.matmul(
                ps[:, s * 512 : (s + 1) * 512],
                kt_[kr : kr + DH, mt * P : (mt + 1) * P],
                qt[qr : qr + DH, w * W + s * 512 : w * W + (s + 1) * 512],
                start=True,
                stop=True,
            )
        return ps

    def expf(ps, mt):
        et = exps.tile([P, W], bf16, tag="exp", name="et")
        nc.scalar.activation(
            et[:], ps[:], mybir.ActivationFunctionType.Exp,
            bias=mb_sb[:, mt : mt + 1], scale=SCALE,
        )
        return et

    # one PV chain: 128 queries (tile j of half w) x [v_h | ones]
    def chain(ets, h, j):
        cps = ch.tile([P, DH + 1], f32, tag="ch", name="cps")
        for mt in range(NMT):
            nc.tensor.matmul(
                cps[:, :],
                ets[mt][:, j * P : (j + 1) * P],
                v_sb[:, mt * VB + h * (DH + 1) : mt * VB + (h + 1) * (DH + 1)],
                start=(mt == 0),
                stop=(mt == NMT - 1),
            )
        return cps

    # normalize chain j of head h into the norm tile for (pair, w)
    def norm(cps, dest_ap):
        rc = rcs.tile([P, 1], f32, tag="rc", name="rc")
        nc.vector.reciprocal(rc[:], cps[:, DH : DH + 1])
        nc.vector.tensor_scalar_mul(dest_ap, cps[:, 0:DH], rc[:])

    def proj(nt):
        ps_y = pa.tile([P, W], f32, tag="pa", name="ps_y")
        for o0, ow in ((0, 512), (512, 256)):
            nc.tensor.matmul(
                ps_y[:, o0 : o0 + ow],
                atA[:, nt * P : (nt + 1) * P],
                wpA[:, o0 : o0 + ow],
                start=True,
                stop=False,
            )
            nc.tensor.matmul(
                ps_y[:, o0 : o0 + ow],
                atB[:, nt * P : (nt + 1) * P],
                wpB[:, o0 : o0 + ow],
                start=False,
                stop=True,
            )
        yt = ys.tile([P, C], bf16, tag="y", name="yt")
        nc.vector.tensor_copy(yt[:], ps_y[:, :C])
        nc.sync.dma_start(y[nt * P : (nt + 1) * P, :], yt[:])

    # ---------------- schedule ----------------
    # units: (head, half); unit u's loop hosts unit u-1's chains as filler
    UNITS = [(0, 0), (1, 0), (2, 0), (0, 1), (1, 1), (2, 1)]

    # prologue: enough q/k for unit 0 (q h0 cols 0:1024, k tiles arrive per mt)
    qk_pass(0, P, 0, qA)
    qk_pass(P, P, 0, kA)
    qk_pass(0, P, 1, qA)

    # filler schedule per unit: list of (mt_slot, callable)
    # passes still needed after prologue, spread over units 0-1
    def mk_pass(col0, rows, c, dest):
        return lambda: qk_pass(col0, rows, c, dest)

    fillers = {u: [] for u in range(len(UNITS))}
    # unit 0: k chunks ahead of their key tiles, q2/k2 chunk 0/1, v tiles
    fillers[0] = [
        (1, mk_pass(2 * P, DH, 0, qB)),
        (2, lambda: v_tile(0)), (2, lambda: v_tile(1)),
        (3, mk_pass(P, P, 1, kA)),       # k tiles 4..7 before mt 4
        (4, lambda: v_tile(2)), (4, lambda: v_tile(3)),
        (5, mk_pass(2 * P + DH, DH, 0, kB)),
        (6, lambda: v_tile(4)), (6, lambda: v_tile(5)),
        (7, mk_pass(P, P, 2, kA)),       # k tiles 8..11 before mt 8
        (8, lambda: v_tile(6)), (8, lambda: v_tile(7)),
        (9, mk_pass(0, P, 2, qA)),       # q w1 chunk
        (10, lambda: v_tile(8)), (10, lambda: v_tile(9)),
        (11, mk_pass(P, P, 3, kA)),      # k tiles 12..15 before mt 12
        (12, lambda: v_tile(10)), (12, lambda: v_tile(11)),
        (13, mk_pass(0, P, 3, qA)),
        (14, lambda: v_tile(12)), (14, lambda: v_tile(13)),
        (15, lambda: v_tile(14)),
    ]
    fillers[1] = [
        (0, lambda: v_tile(15)),
        (1, mk_pass(2 * P, DH, 1, qB)),
        (2, mk_pass(2 * P + DH, DH, 1, kB)),
        (3, mk_pass(2 * P, DH, 2, qB)),
        (4, mk_pass(2 * P + DH, DH, 2, kB)),
        (5, mk_pass(2 * P, DH, 3, qB)),
        (6, mk_pass(2 * P + DH, DH, 3, kB)),
    ]

    norm_tiles = {}

    def norm_dest(h, w, j):
        # pair tile for (h0,h1); own tile for h2 (pad cols stay zero)
        key = ("A" if h < 2 else "B", w)
        if key not in norm_tiles:
            t = norms.tile([P, W], bf16, tag="nt" + key[0], name="ntile")
            if key[0] == "B":
                nc.gpsimd.memset(t[:], 0.0)
            norm_tiles[key] = t
        t = norm_tiles[key]
        off = j * P + (DH if h == 1 else 0)
        return t[:, off : off + DH]

    def transpose_pair(key, w, dest, rows):
        t = norm_tiles.pop((key, w))
        nc.sync.dma_start_transpose(
            dest[0:rows, w * W : (w + 1) * W].rearrange(
                "p (b q) -> p b q", q=P
            ),
            t[:],
        )

    prev = None  # (ets, h) of previous unit
    for u, (h, w) in enumerate(UNITS):
        ets = []
        fill = list(fillers[u])
        for mt in range(NMT):
            ps = logits(h, w, mt)
            ets.append(expf(ps, mt))
            while fill and fill[0][0] <= mt:
                fill.pop(0)[1]()
            if prev is not None and 3 <= mt < 3 + JT:
                ph, pw, pets = prev
                j = mt - 3
                cps = chain(pets, ph, j)
                norm(cps, norm_dest(ph, pw, j))
                if ph == 1 and j == JT - 1:
                    transpose_pair("A", pw, atA, P)
                if ph == 2 and j == JT - 1:
                    transpose_pair("B", pw, atB, DH)
            # proj fillers: half 0 ready during unit 4 (after h2 w0 transpose)
            if u >= 4 and mt % 2 == 1:
                nt = (u - 4) * JT + (mt - 1) // 2
                if nt < JT:
                    proj(nt)
        prev = (h, w, ets)

    # tail: chains of the last unit (h2, w1), then remaining projections
    ph, pw, pets = prev
    for j in range(JT):
        cps = chain(pets, ph, j)
        norm(cps, norm_dest(ph, pw, j))
        if j == JT - 1:
            transpose_pair("B", pw, atB, DH)
    for nt in range(JT, NQT):
        proj(nt)


def _get_nc(reps=1):
    if reps not in _nc_cache:
        _nc_cache[reps] = _build(reps)
    return _nc_cache[reps]


def prep_in_maps(x, att_mask, qkv_w, proj_w):
    """Per-core input prep (host): slice heads, transpose, cast to bf16."""
    in_maps = []
    for b in range(B):
        xT = np.ascontiguousarray(x[b].T).astype(ml_dtypes.bfloat16)
        mb = np.where(att_mask[b] == 0, -1e30, 0.0).astype(np.float32)
        mbias = np.ascontiguousarray(mb.reshape(NMT, P).T)
        for g in range(G):
            r0 = g * HPC * DH
            r1 = (g + 1) * HPC * DH
            wq = qkv_w[r0:r1]
            wk = qkv_w[C + r0 : C + r1]
            wv = qkv_w[2 * C + r0 : 2 * C + r1]
            wqkT = np.ascontiguousarray(
                np.concatenate(
                    [wq[0 : 2 * DH], wk[0 : 2 * DH], wq[2 * DH :], wk[2 * DH :]], 0
                ).T
            ).astype(ml_dtypes.bfloat16)
            wvT = np.ascontiguousarray(wv.T).astype(ml_dtypes.bfloat16)
            wpT = np.ascontiguousarray(proj_w[:, r0:r1].T)
            in_maps.append(
                {
                    "xT": xT,
                    "wqkT": wqkT,
                    "wvT": wvT,
                    "wpTA": wpT[0 : 2 * DH].astype(ml_dtypes.bfloat16),
                    "wpTB": wpT[2 * DH :].astype(ml_dtypes.bfloat16),
                    "mbias": mbias,
                }
            )
    return in_maps


def kernel(x, att_mask, qkv_w, proj_w, proj_b):
    global LAST_EXEC_NS, LAST_RESULTS
    x = np.asarray(x, dtype=np.float32)
    att_mask = np.asarray(att_mask)
    qkv_w = np.asarray(qkv_w, dtype=np.float32)
    proj_w = np.asarray(proj_w, dtype=np.float32)
    proj_b = np.asarray(proj_b, dtype=np.float32)

    nc = _get_nc()
    in_maps = prep_in_maps(x, att_mask, qkv_w, proj_w)

    res = run_bass_kernel_spmd(
        nc, in_maps, core_ids=list(range(B * G)), trace=TRACE
    )
    LAST_EXEC_NS = res.exec_time_ns
    LAST_RESULTS = res

    out = np.zeros((B, N, C), np.float32)
    for b in range(B):
        acc = np.asarray(res.results[b * G]["y"]).astype(np.float32)
        for g in range(1, G):
            acc += np.asarray(res.results[b * G + g]["y"]).astype(np.float32)
        out[b] = acc + proj_b[None, :]
    return out


# revision 9
# speedup vs baseline: 1.1897x; 1.1178x over previous
"""Multi-head attention forward (B=2, N=2048, C=768, H=12) on 8 TRN2 cores.

Sharding: core = b*4 + g handles batch b, heads 3g..3g+2. Each core computes
qkv for its heads (all matmul operands bf16), full N x N logits per head in
[key, query] orientation (key mask folds into the exp bias), exp on the
Activation engine into bf16 SBUF tiles, then a "flipped" PV: the exp tile is
the stationary operand and the moving operand is the 65-wide [v | ones]
block, so each 128-query chain costs 65 cycles/k-tile and the softmax
denominator rides in column 64. Normalization is a per-partition
reciprocal+scale on DVE; a single xbar DMA transpose per (head-pair,
query-half) flips the [query, dim] chains into the d-major layout the output
projection needs. Host sums the 4 per-group partial projections per batch
and adds the bias.

Work is laid out as 6 sequential units (3 heads x 2 query halves). Unit U's
logits/exp loop is interleaved (in PE program order) with unit U-1's PV
chains plus qkv/v/proj filler passes so the PE never waits on the
Activation engine, which is the per-unit long pole.
"""

import numpy as np
import ml_dtypes

from concourse import bacc
import concourse.mybir as mybir
import concourse.tile as tile
from concourse.bass_utils import run_bass_kernel_spmd

B, N, C = 2, 2048, 768
H, DH = 12, 64
G = 4           # head groups (cores per batch)
HPC = 3         # heads per core
P = 128
KT = C // P     # 6 contraction tiles over channels
NMT = N // P    # 16 key tiles
W = 1024        # query-half width
NQT = N // P    # 16 query tiles (128 each)
JT = W // P     # 8 query tiles per half
VB = HPC * (DH + 1)   # 195: per-mt v block [v0|1|v1|1|v2|1]
SCALE = float(DH) ** -0.5

TRACE = False
LAST_EXEC_NS = None
LAST_RESULTS = None

_nc_cache = {}

f32 = mybir.dt.float32
bf16 = mybir.dt.bfloat16


def _build(reps=1):
    nc = bacc.Bacc("TRN2", debug=False)

    xT = nc.dram_tensor("xT", [C, N], bf16, kind="ExternalInput")
    wqkT = nc.dram_tensor("wqkT", [C, 3 * P], bf16, kind="ExternalInput")
    wvT = nc.dram_tensor("wvT", [C, HPC * DH], bf16, kind="ExternalInput")
    wpTA = nc.dram_tensor("wpTA", [P, C], bf16, kind="ExternalInput")
    wpTB = nc.dram_tensor("wpTB", [DH, C], bf16, kind="ExternalInput")
    mbias = nc.dram_tensor("mbias", [P, NMT], f32, kind="ExternalInput")
    y = nc.dram_tensor("y", [N, C], bf16, kind="ExternalOutput")

    with tile.TileContext(nc) as tc:
        with (
            tc.tile_pool(name="big", bufs=1) as big,
            tc.tile_pool(name="exps", bufs=34) as exps,
            tc.tile_pool(name="norms", bufs=2) as norms,
            tc.tile_pool(name="rcs", bufs=6) as rcs,
            tc.tile_pool(name="ys", bufs=4) as ys,
            tc.tile_pool(name="pa", bufs=2, space="PSUM") as pa,
            tc.tile_pool(name="ch", bufs=4, space="PSUM") as ch,
        ):
            for _ in range(reps):
                body(nc, tc, big, exps, norms, rcs, ys, pa, ch,
                     xT, wqkT, wvT, wpTA, wpTB, mbias, y)

    nc.compile()
    return nc


def body(nc, tc, big, exps, norms, rcs, ys, pa, ch,
         xT, wqkT, wvT, wpTA, wpTB, mbias, y):
    xT_sb = big.tile([P, KT * N], bf16, tag="xT", name="xT_sb")
    wqk_sb = big.tile([P, KT * 3 * P], bf16, tag="wqk", name="wqk_sb")
    wv_sb = big.tile([P, KT * HPC * DH], bf16, tag="wv", name="wv_sb")
    wpA = big.tile([P, C], bf16, tag="wpA", name="wpA")
    wpB = big.tile([DH, C], bf16, tag="wpB", name="wpB")
    mb_sb = big.tile([P, NMT], f32, tag="mb", name="mb_sb")
    qA = big.tile([P, N], bf16, tag="qA", name="qA")    # q d-major h0|h1
    kA = big.tile([P, N], bf16, tag="kA", name="kA")    # k d-major h0|h1
    qB = big.tile([DH, N], bf16, tag="qB", name="qB")   # q d-major h2
    kB = big.tile([DH, N], bf16, tag="kB", name="kB")   # k d-major h2
    v_sb = big.tile([P, NMT * VB], bf16, tag="v", name="v_sb")
    atA = big.tile([P, N], bf16, tag="atA", name="atA")  # d-major attn h0|h1
    atB = big.tile([DH, N], bf16, tag="atB", name="atB")  # d-major attn h2

    # --- input DMAs, ordered by first use (wqk + x chunks 0/1 gate unit 0) ---
    xTv = xT[:, :].rearrange("(k p) n -> p k n", p=P)
    xsv = xT_sb[:].rearrange("p (k n) -> p k n", n=N)

    def dma_x(c):
        nc.sync.dma_start(
            xsv[:, :, c * 512 : (c + 1) * 512], xTv[:, :, c * 512 : (c + 1) * 512]
        )

    nc.sync.dma_start(
        wqk_sb[:].rearrange("p (k c) -> p k c", c=3 * P),
        wqkT[:, :].rearrange("(k p) c -> p k c", p=P),
    )
    dma_x(0)
    dma_x(1)
    nc.sync.dma_start(mb_sb[:], mbias[:, :])
    dma_x(2)
    dma_x(3)
    nc.sync.dma_start(
        wv_sb[:].rearrange("p (k c) -> p k c", c=HPC * DH),
        wvT[:, :].rearrange("(k p) c -> p k c", p=P),
    )
    nc.sync.dma_start(wpA[:], wpTA[:, :])
    nc.sync.dma_start(wpB[:], wpTB[:, :])

    # ones columns of the v blocks (static); warmup source tile
    ones_w = big.tile([P, 512], bf16, tag="onesw", name="ones_w")
    nc.gpsimd.memset(ones_w[:], 1.0)
    nc.gpsimd.memset(
        v_sb[:].rearrange("p (a c) -> p a c", c=DH + 1)[:, :, DH : DH + 1], 1.0
    )

    # keep the PE busy while input DMAs land so the p-state ramp finishes
    # before real work starts (results are never read)
    def warmup(n):
        ps = pa.tile([P, 512], f32, tag="pa", name="ps_warm")
        for _ in range(n):
            nc.tensor.matmul(
                ps[:, :], ones_w[:, 0:P], ones_w[:, :], start=True, stop=True
            )

    # --- qk pass: d-major q/k for one 512-query chunk ---
    # wqk col blocks: [wq_h0|wq_h1][wk_h0|wk_h1][wq_h2|wk_h2]
    def qk_pass(col0, rows, c, dest):
        ps = pa.tile([P, 512], f32, tag="pa", name="ps_qk")
        for kt in range(KT):
            nc.tensor.matmul(
                ps[0:rows, :],
                wqk_sb[:, kt * 3 * P + col0 : kt * 3 * P + col0 + rows],
                xT_sb[:, kt * N + c * 512 : kt * N + (c + 1) * 512],
                start=(kt == 0),
                stop=(kt == KT - 1),
            )
        nc.vector.tensor_copy(dest[:, c * 512 : (c + 1) * 512], ps[0:rows, :])

    # --- v pass: keys-major v for one 128-key tile ---
    def v_tile(mt):
        ps = pa.tile([P, HPC * DH], f32, tag="pa", name="ps_v")
        for kt in range(KT):
            nc.tensor.matmul(
                ps[:, :],
                xT_sb[:, kt * N + mt * P : kt * N + (mt + 1) * P],
                wv_sb[:, kt * HPC * DH : (kt + 1) * HPC * DH],
                start=(kt == 0),
                stop=(kt == KT - 1),
            )
        nc.vector.tensor_copy(
            v_sb[:].rearrange("p (m a c) -> p (m a) c", c=DH + 1, a=HPC)[
                :, mt * HPC : (mt + 1) * HPC, 0:DH
            ],
            ps[:].rearrange("p (a c) -> p a c", c=DH),
        )

    # q/k APs per head: (tile, row0)
    QAP = {0: (qA, 0), 1: (qA, DH), 2: (qB, 0)}
    KAP = {0: (kA, 0), 1: (kA, DH), 2: (kB, 0)}

    def logits(h, w, mt):
        qt, qr = QAP[h]
        kt_, kr = KAP[h]
        ps = pa.tile([P, W], f32, tag="pa", name="ps_l")
        for s in range(W // 512):
            nc.tensor.matmul(
                ps[:, s * 512 : (s + 1) * 512],
                kt_[kr : kr + DH, mt * P : (mt + 1) * P],
                qt[qr : qr + DH, w * W + s * 512 : w * W + (s + 1) * 512],
                start=True,
                stop=True,
            )
        return ps

    def expf(ps, mt):
        et = exps.tile([P, W], bf16, tag="exp", name="et")
        nc.scalar.activation(
            et[:], ps[:], mybir.ActivationFunctionType.Exp,
            bias=mb_sb[:, mt : mt + 1], scale=SCALE,
        )
        return et

    # one PV chain: 128 queries (tile j of half w) x [v_h | ones]
    def chain(ets, h, j):
        cps = ch.tile([P, DH + 1], f32, tag="ch", name="cps")
        for mt in range(NMT):
            nc.tensor.matmul(
                cps[:, :],
                ets[mt][:, j * P : (j + 1) * P],
                v_sb[:, mt * VB + h * (DH + 1) : mt * VB + (h + 1) * (DH + 1)],
                start=(mt == 0),
                stop=(mt == NMT - 1),
            )
        return cps

    # normalize chain j of head h into the norm tile for (pair, w)
    def norm(cps, dest_ap):
        rc = rcs.tile([P, 1], f32, tag="rc", name="rc")
        nc.vector.reciprocal(rc[:], cps[:, DH : DH + 1])
        nc.vector.tensor_scalar_mul(dest_ap, cps[:, 0:DH], rc[:])

    def proj(nt):
        ps_y = pa.tile([P, W], f32, tag="pa", name="ps_y")
        for o0, ow in ((0, 512), (512, 256)):
            nc.tensor.matmul(
                ps_y[:, o0 : o0 + ow],
                atA[:, nt * P : (nt + 1) * P],
                wpA[:, o0 : o0 + ow],
                start=True,
                stop=False,
            )
            nc.tensor.matmul(
                ps_y[:, o0 : o0 + ow],
                atB[:, nt * P : (nt + 1) * P],
                wpB[:, o0 : o0 + ow],
                start=False,
                stop=True,
            )
        yt = ys.tile([P, C], bf16, tag="y", name="yt")
        nc.vector.tensor_copy(yt[:], ps_y[:, :C])
        nc.sync.dma_start(y[nt * P : (nt + 1) * P, :], yt[:])

    # ---------------- schedule ----------------
    # units: (head, half); unit u's loop hosts unit u-1's chains as filler
    UNITS = [(0, 0), (1, 0), (2, 0), (0, 1), (1, 1), (2, 1)]
    CH0 = 6  # chains of the previous unit run at mt slots CH0..CH0+7

    warmup(10)
    # prologue: enough q/k for unit 0 (q h0 cols 0:1024, k tiles per chunk)
    qk_pass(0, P, 0, qA)
    qk_pass(P, P, 0, kA)
    qk_pass(0, P, 1, qA)

    def mk_pass(col0, rows, c, dest):
        return lambda: qk_pass(col0, rows, c, dest)

    fillers = {u: [] for u in range(len(UNITS))}
    # unit 0: k chunks ahead of their key tiles, v tiles, h2 chunk 0
    fillers[0] = [
        (1, lambda: v_tile(0)),
        (2, lambda: v_tile(1)),
        (3, mk_pass(P, P, 1, kA)),       # k tiles 4..7 before mt 4
        (4, lambda: v_tile(2)),
        (5, lambda: v_tile(3)),
        (6, lambda: v_tile(4)),
        (7, mk_pass(P, P, 2, kA)),       # k tiles 8..11 before mt 8
        (8, lambda: v_tile(5)),
        (9, lambda: v_tile(6)),
        (10, lambda: v_tile(7)),
        (11, mk_pass(P, P, 3, kA)),      # k tiles 12..15 before mt 12
        (12, lambda: v_tile(8)),
        (13, lambda: v_tile(9)),
        (14, mk_pass(2 * P, DH, 0, qB)),
        (15, mk_pass(2 * P + DH, DH, 0, kB)),
    ]
    fillers[1] = [
        (0, lambda: v_tile(10)),
        (1, lambda: v_tile(11)),
        (2, lambda: v_tile(12)),
        (3, lambda: v_tile(13)),
        (4, lambda: v_tile(14)),
        (5, lambda: v_tile(15)),
        (8, mk_pass(0, P, 2, qA)),       # q half 1 for unit 3
        (12, mk_pass(2 * P, DH, 1, qB)),
    ]
    fillers[2] = [
        (0, mk_pass(2 * P + DH, DH, 1, kB)),   # k2 tiles 4..7 before mt 4
        (2, mk_pass(2 * P + DH, DH, 2, kB)),   # before mt 8
        (4, mk_pass(2 * P + DH, DH, 3, kB)),   # before mt 12
        (8, mk_pass(0, P, 3, qA)),
        (12, mk_pass(2 * P, DH, 2, qB)),
        (14, mk_pass(2 * P, DH, 3, qB)),
    ]
    # proj(0..7) gated on the w0 transposes (end of unit 3's chain hosting)
    fillers[3] = [(14, lambda: proj(0)), (15, lambda: proj(1))]
    fillers[4] = [
        (0, lambda: proj(2)), (2, lambda: proj(3)), (4, lambda: proj(4)),
        (6, lambda: proj(5)), (8, lambda: proj(6)), (10, lambda: proj(7)),
    ]

    norm_tiles = {}

    def norm_dest(h, w, j):
        # pair tile for (h0,h1); own tile for h2 (pad cols stay zero)
        key = ("A" if h < 2 else "B", w)
        if key not in norm_tiles:
            t = norms.tile([P, W], bf16, tag="nt" + key[0], name="ntile")
            if key[0] == "B":
                nc.gpsimd.memset(t[:], 0.0)
            norm_tiles[key] = t
        t = norm_tiles[key]
        off = j * P + (DH if h == 1 else 0)
        return t[:, off : off + DH]

    def transpose_cols(key, w, dest, rows, c0, c1):
        t = norm_tiles[(key, w)]
        nc.sync.dma_start_transpose(
            dest[0:rows, w * W + c0 : w * W + c1].rearrange(
                "p (b q) -> p b q", q=P
            ),
            t[:, c0:c1],
        )

    def transpose_pair(key, w, dest, rows):
        transpose_cols(key, w, dest, rows, 0, W)
        del norm_tiles[(key, w)]

    prev = None  # (head, half, ets) of previous unit
    for u, (h, w) in enumerate(UNITS):
        ets = []
        fill = list(fillers[u])
        for mt in range(NMT):
            ps = logits(h, w, mt)
            ets.append(expf(ps, mt))
            while fill and fill[0][0] <= mt:
                fill.pop(0)[1]()
            if prev is not None and CH0 <= mt < CH0 + JT:
                ph, pw, pets = prev
                j = mt - CH0
                cps = chain(pets, ph, j)
                norm(cps, norm_dest(ph, pw, j))
                if ph == 1 and j == JT - 1:
                    transpose_pair("A", pw, atA, P)
                if ph == 2 and j == JT - 1:
                    transpose_pair("B", pw, atB, DH)
        prev = (h, w, ets)

    # tail: chains of the last unit (h2, w1), pipelined per q-tile pair with
    # mini transposes so projections start as soon as their tile is ready
    ph, pw, pets = prev
    for j in range(JT):
        cps = chain(pets, ph, j)
        norm(cps, norm_dest(ph, pw, j))
        if j % 2 == 1:
            transpose_cols("B", pw, atB, DH, (j - 1) * P, (j + 1) * P)
            proj(JT + j - 1)
            proj(JT + j)
    del norm_tiles[("B", pw)]


def _get_nc(reps=1):
    if reps not in _nc_cache:
        _nc_cache[reps] = _build(reps)
    return _nc_cache[reps]


def prep_in_maps(x, att_mask, qkv_w, proj_w):
    """Per-core input prep (host): slice heads, transpose, cast to bf16."""
    in_maps = []
    for b in range(B):
        xT = np.ascontiguousarray(x[b].T).astype(ml_dtypes.bfloat16)
        mb = np.where(att_mask[b] == 0, -1e30, 0.0).astype(np.float32)
        mbias = np.ascontiguousarray(mb.reshape(NMT, P).T)
        for g in range(G):
            r0 = g * HPC * DH
            r1 = (g + 1) * HPC * DH
            wq = qkv_w[r0:r1]
            wk = qkv_w[C + r0 : C + r1]
            wv = qkv_w[2 * C + r0 : 2 * C + r1]
            wqkT = np.ascontiguousarray(
                np.concatenate(
                    [wq[0 : 2 * DH], wk[0 : 2 * DH], wq[2 * DH :], wk[2 * DH :]], 0
                ).T
            ).astype(ml_dtypes.bfloat16)
            wvT = np.ascontiguousarray(wv.T).astype(ml_dtypes.bfloat16)
            wpT = np.ascontiguousarray(proj_w[:, r0:r1].T)
            in_maps.append(
                {
                    "xT": xT,
                    "wqkT": wqkT,
                    "wvT": wvT,
                    "wpTA": wpT[0 : 2 * DH].astype(ml_dtypes.bfloat16),
                    "wpTB": wpT[2 * DH :].astype(ml_dtypes.bfloat16),
                    "mbias": mbias,
                }
            )
    return in_maps


def kernel(x, att_mask, qkv_w, proj_w, proj_b):
    global LAST_EXEC_NS, LAST_RESULTS
    x = np.asarray(x, dtype=np.float32)
    att_mask = np.asarray(att_mask)
    qkv_w = np.asarray(qkv_w, dtype=np.float32)
    proj_w = np.asarray(proj_w, dtype=np.float32)
    proj_b = np.asarray(proj_b, dtype=np.float32)

    nc = _get_nc()
    in_maps = prep_in_maps(x, att_mask, qkv_w, proj_w)

    res = run_bass_kernel_spmd(
        nc, in_maps, core_ids=list(range(B * G)), trace=TRACE
    )
    LAST_EXEC_NS = res.exec_time_ns
    LAST_RESULTS = res

    out = np.zeros((B, N, C), np.float32)
    for b in range(B):
        acc = np.asarray(res.results[b * G]["y"]).astype(np.float32)
        for g in range(1, G):
            acc += np.asarray(res.results[b * G + g]["y"]).astype(np.float32)
        out[b] = acc + proj_b[None, :]
    return out


# revision 18
# speedup vs baseline: 1.3337x; 1.1211x over previous
"""Multi-head attention forward (B=2, N=2048, C=768, H=12) on 8 TRN2 cores.

Sharding: core = b*4 + g handles batch b, heads 3g..3g+2. Each core computes
qkv for its heads (all matmul operands bf16), full N x N logits per head in
[key, query] orientation (key mask folds into the exp bias), exp on the
Activation engine into bf16 SBUF tiles, then a "flipped" PV: the exp tile is
the stationary operand and the moving operand is the 65-wide [v | ones]
block, so each 128-query chain costs 65 cycles/k-tile and the softmax
denominator rides in column 64. Normalization is a per-partition
reciprocal+scale on DVE; a single xbar DMA transpose per (head-pair,
query-half) flips the [query, dim] chains into the d-major layout the output
projection needs. Host sums the 4 per-group partial projections per batch
and adds the bias.

Work is laid out as 6 sequential units (3 heads x 2 query halves). Unit U's
logits/exp loop is interleaved (in PE program order) with unit U-1's PV
chains plus qkv/v/proj filler passes so the PE never waits on the
Activation engine, which is the per-unit long pole.
"""

import numpy as np
import ml_dtypes

from concourse import bacc
import concourse.mybir as mybir
import concourse.tile as tile
from concourse.bass_utils import run_bass_kernel_spmd

B, N, C = 2, 2048, 768
H, DH = 12, 64
G = 4           # head groups (cores per batch)
HPC = 3         # heads per core
P = 128
KT = C // P     # 6 contraction tiles over channels
NMT = N // P    # 16 key tiles
W = 1024        # query-half width
NQT = N // P    # 16 query tiles (128 each)
JT = W // P     # 8 query tiles per half
VB = HPC * (DH + 1)   # 195: per-mt v block [v0|1|v1|1|v2|1]
SCALE = float(DH) ** -0.5

TRACE = False
LAST_EXEC_NS = None
LAST_RESULTS = None

_nc_cache = {}

f32 = mybir.dt.float32
bf16 = mybir.dt.bfloat16


def _build(reps=1):
    nc = bacc.Bacc("TRN2", debug=False)

    xT = nc.dram_tensor("xT", [C, N], bf16, kind="ExternalInput")
    wqkT = nc.dram_tensor("wqkT", [C, 3 * P], bf16, kind="ExternalInput")
    wvT = nc.dram_tensor("wvT", [C, HPC * DH], bf16, kind="ExternalInput")
    wpTA = nc.dram_tensor("wpTA", [P, C], bf16, kind="ExternalInput")
    wpTB = nc.dram_tensor("wpTB", [DH, C], bf16, kind="ExternalInput")
    mbias = nc.dram_tensor("mbias", [P, NMT], f32, kind="ExternalInput")
    y = nc.dram_tensor("y", [N, C], bf16, kind="ExternalOutput")

    with tile.TileContext(nc) as tc:
        with (
            tc.tile_pool(name="big", bufs=1) as big,
            tc.tile_pool(name="exps", bufs=34) as exps,
            tc.tile_pool(name="norms", bufs=2) as norms,
            tc.tile_pool(name="rcs", bufs=6) as rcs,
            tc.tile_pool(name="ys", bufs=4) as ys,
            tc.tile_pool(name="pa", bufs=3, space="PSUM") as pa,
            tc.tile_pool(name="ch", bufs=2, space="PSUM") as ch,
        ):
            for _ in range(reps):
                body(nc, tc, big, exps, norms, rcs, ys, pa, ch,
                     xT, wqkT, wvT, wpTA, wpTB, mbias, y)

    nc.compile()
    return nc


def body(nc, tc, big, exps, norms, rcs, ys, pa, ch,
         xT, wqkT, wvT, wpTA, wpTB, mbias, y):
    xT_sb = big.tile([P, KT * N], bf16, tag="xT", name="xT_sb")
    wqk_sb = big.tile([P, KT * 3 * P], bf16, tag="wqk", name="wqk_sb")
    wv_sb = big.tile([P, KT * HPC * DH], bf16, tag="wv", name="wv_sb")
    wpA = big.tile([P, C], bf16, tag="wpA", name="wpA")
    wpB = big.tile([DH, C], bf16, tag="wpB", name="wpB")
    mb_sb = big.tile([P, NMT], f32, tag="mb", name="mb_sb")
    qA = big.tile([P, N], bf16, tag="qA", name="qA")    # q d-major h0|h1
    kA = big.tile([P, N], bf16, tag="kA", name="kA")    # k d-major h0|h1
    tB = big.tile([P, N], bf16, tag="tB", name="tB")    # q2 (0:64) | k2 (64:128)
    kB = big.tile([DH, N], bf16, tag="kB", name="kB")   # k2 moved to base 0
    v_sb = big.tile([P, NMT * VB], bf16, tag="v", name="v_sb")
    atA = big.tile([P, N], bf16, tag="atA", name="atA")  # d-major attn h0|h1
    atB = big.tile([DH, N], bf16, tag="atB", name="atB")  # d-major attn h2

    # --- input DMAs, ordered by first use (wqk + x chunks 0/1 gate unit 0) ---
    xTv = xT[:, :].rearrange("(k p) n -> p k n", p=P)
    xsv = xT_sb[:].rearrange("p (k n) -> p k n", n=N)

    def dma_x(c):
        nc.sync.dma_start(
            xsv[:, :, c * 512 : (c + 1) * 512], xTv[:, :, c * 512 : (c + 1) * 512]
        )

    wqkv = wqk_sb[:].rearrange("p (k c) -> p k c", c=3 * P)
    wqkTv = wqkT[:, :].rearrange("(k p) c -> p k c", p=P)
    nc.sync.dma_start(wqkv[:, :, 0 : 2 * P], wqkTv[:, :, 0 : 2 * P])
    dma_x(0)
    dma_x(1)
    nc.sync.dma_start(mb_sb[:], mbias[:, :])
    nc.sync.dma_start(wqkv[:, :, 2 * P : 3 * P], wqkTv[:, :, 2 * P : 3 * P])
    dma_x(2)
    dma_x(3)
    nc.sync.dma_start(
        wv_sb[:].rearrange("p (k c) -> p k c", c=HPC * DH),
        wvT[:, :].rearrange("(k p) c -> p k c", p=P),
    )
    nc.sync.dma_start(wpA[:], wpTA[:, :])
    nc.sync.dma_start(wpB[:], wpTB[:, :])

    # ones columns of the v blocks (static); warmup source tile
    ones_w = big.tile([P, 512], bf16, tag="onesw", name="ones_w")
    nc.gpsimd.memset(ones_w[:], 1.0)
    nc.gpsimd.memset(
        v_sb[:].rearrange("p (a c) -> p a c", c=DH + 1)[:, :, DH : DH + 1], 1.0
    )

    # keep the PE busy while input DMAs land so the p-state ramp finishes
    # before real work starts (results are never read)
    def warmup(n):
        ps = pa.tile([P, 512], f32, tag="pa", name="ps_warm")
        for _ in range(n):
            nc.tensor.matmul(
                ps[:, :], ones_w[:, 0:P], ones_w[:, :], start=True, stop=True
            )

    # --- qk pass: d-major q/k for one 512-query chunk ---
    # wqk col blocks: [wq_h0|wq_h1][wk_h0|wk_h1][wq_h2|wk_h2]
    def qk_pass(col0, rows, c, dest):
        ps = pa.tile([P, 512], f32, tag="pa", name="ps_qk")
        for kt in range(KT):
            nc.tensor.matmul(
                ps[0:rows, :],
                wqk_sb[:, kt * 3 * P + col0 : kt * 3 * P + col0 + rows],
                xT_sb[:, kt * N + c * 512 : kt * N + (c + 1) * 512],
                start=(kt == 0),
                stop=(kt == KT - 1),
            )
        nc.vector.tensor_copy(dest[:, c * 512 : (c + 1) * 512], ps[0:rows, :])

    # --- v pass: keys-major v for one 128-key tile ---
    def v_tile(mt):
        ps = pa.tile([P, HPC * DH], f32, tag="pa", name="ps_v")
        for kt in range(KT):
            nc.tensor.matmul(
                ps[:, :],
                xT_sb[:, kt * N + mt * P : kt * N + (mt + 1) * P],
                wv_sb[:, kt * HPC * DH : (kt + 1) * HPC * DH],
                start=(kt == 0),
                stop=(kt == KT - 1),
            )
        nc.vector.tensor_copy(
            v_sb[:].rearrange("p (m a c) -> p (m a) c", c=DH + 1, a=HPC)[
                :, mt * HPC : (mt + 1) * HPC, 0:DH
            ],
            ps[:].rearrange("p (a c) -> p a c", c=DH),
        )

    # q/k APs per head: (tile, row0)
    QAP = {0: (qA, 0), 1: (qA, DH), 2: (tB, 0)}
    KAP = {0: (kA, 0), 1: (kA, DH), 2: (kB, 0)}

    def logits(h, w, mt):
        qt, qr = QAP[h]
        kt_, kr = KAP[h]
        ps = pa.tile([P, W], f32, tag="pa", name="ps_l")
        for s in range(W // 512):
            nc.tensor.matmul(
                ps[:, s * 512 : (s + 1) * 512],
                kt_[kr : kr + DH, mt * P : (mt + 1) * P],
                qt[qr : qr + DH, w * W + s * 512 : w * W + (s + 1) * 512],
                start=True,
                stop=True,
            )
        return ps

    def expf(ps, mt):
        et = exps.tile([P, W], bf16, tag="exp", name="et")
        nc.scalar.activation(
            et[:], ps[:], mybir.ActivationFunctionType.Exp,
            bias=mb_sb[:, mt : mt + 1], scale=SCALE,
        )
        return et

    # one PV chain: 128 queries (tile j of half w) x [v_h | ones]
    def chain(ets, h, j):
        cps = ch.tile([P, DH + 1], f32, tag="ch", name="cps")
        for mt in range(NMT):
            nc.tensor.matmul(
                cps[:, :],
                ets[mt][:, j * P : (j + 1) * P],
                v_sb[:, mt * VB + h * (DH + 1) : mt * VB + (h + 1) * (DH + 1)],
                start=(mt == 0),
                stop=(mt == NMT - 1),
            )
        return cps

    # normalize chain j of head h into the norm tile for (pair, w)
    def norm(cps, dest_ap):
        rc = rcs.tile([P, 1], f32, tag="rc", name="rc")
        nc.vector.reciprocal(rc[:], cps[:, DH : DH + 1])
        nc.vector.tensor_scalar_mul(dest_ap, cps[:, 0:DH], rc[:])

    def proj(nt, copy_eng=None):
        ps_y = pa.tile([P, W], f32, tag="pa", name="ps_y")
        for o0, ow in ((0, 512), (512, 256)):
            nc.tensor.matmul(
                ps_y[:, o0 : o0 + ow],
                atA[:, nt * P : (nt + 1) * P],
                wpA[:, o0 : o0 + ow],
                start=True,
                stop=False,
            )
            nc.tensor.matmul(
                ps_y[:, o0 : o0 + ow],
                atB[:, nt * P : (nt + 1) * P],
                wpB[:, o0 : o0 + ow],
                start=False,
                stop=True,
            )
        yt = ys.tile([P, C], bf16, tag="y", name="yt")
        if copy_eng == "act":
            nc.scalar.copy(yt[:], ps_y[:, :C])
        else:
            nc.vector.tensor_copy(yt[:], ps_y[:, :C])
        nc.sync.dma_start(y[nt * P : (nt + 1) * P, :], yt[:])

    # ---------------- schedule ----------------
    # units: (head, half); unit u's loop hosts unit u-1's chains as filler
    UNITS = [(0, 0), (1, 0), (2, 0), (0, 1), (1, 1), (2, 1)]
    CH0 = 4  # chains of the previous unit run at mt slots CH0..CH0+7

    warmup(11)
    # prologue: enough q/k for unit 0 (q h0 cols 0:1024, k tiles per chunk)
    qk_pass(0, P, 0, qA)
    qk_pass(P, P, 0, kA)
    qk_pass(0, P, 1, qA)

    def mk_pass(col0, rows, c, dest):
        return lambda: qk_pass(col0, rows, c, dest)

    def kb_dma(c):
        # move k2 of chunk c from tB rows 64:128 to kB rows 0:64
        return lambda: nc.sync.dma_start(
            kB[:, c * 512 : (c + 1) * 512], tB[DH:P, c * 512 : (c + 1) * 512]
        )

    fillers = {u: [] for u in range(len(UNITS))}
    # units 0-1: remaining k chunks (ahead of their key tiles), the combined
    # q2|k2 chunks + k2 base moves, and all v tiles (before unit 0's chains)
    fillers[0] = [
        (1, mk_pass(P, P, 1, kA)),       # k tiles 4..7 before mt 4
        (2, lambda: v_tile(0)),
        (3, lambda: v_tile(1)),
        (4, mk_pass(2 * P, P, 0, tB)),
        (5, kb_dma(0), lambda: v_tile(2)),
        (6, mk_pass(P, P, 2, kA)),       # k tiles 8..11 before mt 8
        (7, lambda: v_tile(3)),
        (8, mk_pass(2 * P, P, 1, tB)),
        (9, kb_dma(1), lambda: v_tile(4)),
        (10, mk_pass(P, P, 3, kA)),      # k tiles 12..15 before mt 12
        (11, lambda: v_tile(5)),
        (12, mk_pass(2 * P, P, 2, tB)),
        (13, kb_dma(2), lambda: v_tile(6)),
        (14, lambda: v_tile(7)),
        (15, lambda: v_tile(8)),
    ]
    fillers[1] = [
        (0, mk_pass(2 * P, P, 3, tB)),
        (1, kb_dma(3), lambda: v_tile(9)),
        (2, lambda: v_tile(10), lambda: v_tile(11)),
        (3, lambda: v_tile(12), lambda: v_tile(13), lambda: v_tile(14),
            lambda: v_tile(15)),
        (13, mk_pass(0, P, 2, qA)),      # q half 1 for unit 3
    ]
    fillers[2] = [
        (0, mk_pass(0, P, 3, qA)),
    ]
    # proj(0..7) gated on the w0 transposes (A at u2-mt11, B at u3-mt11)
    fillers[3] = [
        (13, lambda: proj(0)), (14, lambda: proj(1)), (15, lambda: proj(2)),
    ]
    fillers[4] = [
        (0, lambda: proj(3)), (1, lambda: proj(4)), (2, lambda: proj(5)),
        (3, lambda: proj(6)), (12, lambda: proj(7)),
    ]

    norm_tiles = {}

    def norm_dest(h, w, j):
        # pair tile for (h0,h1); own tile for h2 (pad cols stay zero)
        key = ("A" if h < 2 else "B", w)
        if key not in norm_tiles:
            t = norms.tile([P, W], bf16, tag="nt" + key[0], name="ntile")
            if key[0] == "B":
                nc.gpsimd.memset(t[:], 0.0)
            norm_tiles[key] = t
        t = norm_tiles[key]
        off = j * P + (DH if h == 1 else 0)
        return t[:, off : off + DH]

    def transpose_cols(key, w, dest, rows, c0, c1):
        t = norm_tiles[(key, w)]
        nc.sync.dma_start_transpose(
            dest[0:rows, w * W + c0 : w * W + c1].rearrange(
                "p (b q) -> p b q", q=P
            ),
            t[:, c0:c1],
        )

    def transpose_pair(key, w, dest, rows):
        transpose_cols(key, w, dest, rows, 0, W)
        del norm_tiles[(key, w)]

    prev = None  # (head, half, ets) of previous unit
    for u, (h, w) in enumerate(UNITS):
        ets = []
        fill = list(fillers[u])
        for mt in range(NMT):
            ps = logits(h, w, mt)
            ets.append(expf(ps, mt))
            while fill and fill[0][0] <= mt:
                for fn in fill.pop(0)[1:]:
                    fn()
            if prev is not None and CH0 <= mt < CH0 + JT:
                ph, pw, pets = prev
                j = mt - CH0
                cps = chain(pets, ph, j)
                norm(cps, norm_dest(ph, pw, j))
                if ph == 1 and j == JT - 1:
                    transpose_pair("A", pw, atA, P)
                if ph == 2 and j == JT - 1:
                    transpose_pair("B", pw, atB, DH)
        prev = (h, w, ets)

    # tail: chains of the last unit (h2, w1), pipelined per q-tile pair with
    # mini transposes so projections start as soon as their tile is ready
    ph, pw, pets = prev
    for j in range(JT):
        cps = chain(pets, ph, j)
        norm(cps, norm_dest(ph, pw, j))
        if j % 2 == 1:
            transpose_cols("B", pw, atB, DH, (j - 1) * P, (j + 1) * P)
            proj(JT + j - 1, copy_eng="act")
            proj(JT + j, copy_eng="vec" if j == JT - 1 else "act")
    del norm_tiles[("B", pw)]


def _get_nc(reps=1):
    if reps not in _nc_cache:
        _nc_cache[reps] = _build(reps)
    return _nc_cache[reps]


def prep_in_maps(x, att_mask, qkv_w, proj_w):
    """Per-core input prep (host): slice heads, transpose, cast to bf16."""
    in_maps = []
    for b in range(B):
        xT = np.ascontiguousarray(x[b].T).astype(ml_dtypes.bfloat16)
        mb = np.where(att_mask[b] == 0, -1e30, 0.0).astype(np.float32)
        mbias = np.ascontiguousarray(mb.reshape(NMT, P).T)
        for g in range(G):
            r0 = g * HPC * DH
            r1 = (g + 1) * HPC * DH
            wq = qkv_w[r0:r1]
            wk = qkv_w[C + r0 : C + r1]
            wv = qkv_w[2 * C + r0 : 2 * C + r1]
            wqkT = np.ascontiguousarray(
                np.concatenate(
                    [wq[0 : 2 * DH], wk[0 : 2 * DH], wq[2 * DH :], wk[2 * DH :]], 0
                ).T
            ).astype(ml_dtypes.bfloat16)
            wvT = np.ascontiguousarray(wv.T).astype(ml_dtypes.bfloat16)
            wpT = np.ascontiguousarray(proj_w[:, r0:r1].T)
            in_maps.append(
                {
                    "xT": xT,
                    "wqkT": wqkT,
                    "wvT": wvT,
                    "wpTA": wpT[0 : 2 * DH].astype(ml_dtypes.bfloat16),
                    "wpTB": wpT[2 * DH :].astype(ml_dtypes.bfloat16),
                    "mbias": mbias,
                }
            )
    return in_maps


def kernel(x, att_mask, qkv_w, proj_w, proj_b):
    global LAST_EXEC_NS, LAST_RESULTS
    x = np.asarray(x, dtype=np.float32)
    att_mask = np.asarray(att_mask)
    qkv_w = np.asarray(qkv_w, dtype=np.float32)
    proj_w = np.asarray(proj_w, dtype=np.float32)
    proj_b = np.asarray(proj_b, dtype=np.float32)

    nc = _get_nc()
    in_maps = prep_in_maps(x, att_mask, qkv_w, proj_w)

    res = run_bass_kernel_spmd(
        nc, in_maps, core_ids=list(range(B * G)), trace=TRACE
    )
    LAST_EXEC_NS = res.exec_time_ns
    LAST_RESULTS = res

    out = np.zeros((B, N, C), np.float32)
    for b in range(B):
        acc = np.asarray(res.results[b * G]["y"]).astype(np.float32)
        for g in range(1, G):
            acc += np.asarray(res.results[b * G + g]["y"]).astype(np.float32)
        out[b] = acc + proj_b[None, :]
    return out
